# revision 2
# baseline (speedup 1.0000x reference)
"""HardAttention kernel for Trainium2 (8 NeuronCores, Bass/Tile).

reference:
    scores = einsum("btd,bcsd->btcs", xs, ys)   # (B,Tx,C,Ty)
    out    = scores.max(-1).sum(1)              # (B,C)

Shapes: B=16, Tx=128, C=64, Ty=128, d=768.

Strategy (v2):
  - Data-parallel over B: core i handles batches [2i, 2i+2).
  - Host pre-casts both operands to bf16 and lays them out d-major so the
    PE (which contracts over the partition axis) consumes them directly and
    every DMA is a plain HWDGE copy with large contiguous per-partition
    runs (no SWDGE cast, no strided gather):
        xsP[dk, b, k, t]    = xs[b, t, 128k+dk]        (128, B, 6, Tx)
        ysP[b, dk, c, k, s] = ys[b, c, s, 128k+dk]     (B, 128, C, 6, Ty)
  - Per (b, eighth-of-64-candidates): one 1.6 MB HWDGE DMA (12 KB
    contiguous per partition), then 2 candidate-groups x 6 accumulating
    matmuls (N=512, bf16) into 2 PSUM banks; DVE reduce_max over Ty into
    an SBUF tile m[t, (b,c)]; finally one ones-vector fp32 matmul
    contracts the partition axis (sum over t) -> out[b, c].
"""

import numpy as np

B, TX, C, TY, D = 16, 128, 64, 128, 768
N_CORES = 8
BPC = B // N_CORES          # batches per core = 2
KC = D // 128               # contraction chunks = 6
QC = 8                      # candidates per DMA slab
NQ = C // QC                # slabs per batch = 8
G = 4                       # candidates per matmul (N = G*TY = 512)

_CACHE = {}


def _build():
    import concourse.bass as bass
    import concourse.mybir as mybir
    import concourse.tile as tile
    from concourse import bacc

    bf16 = mybir.dt.bfloat16
    f32 = mybir.dt.float32

    nc = bacc.Bacc(
        "TRN2",
        target_bir_lowering=False,
        debug=False,
        num_devices=N_CORES,
    )

    xs_ap = nc.dram_tensor("xsP", (128, BPC, KC, TX), bf16, kind="ExternalInput").ap()
    ys_ap = nc.dram_tensor(
        "ysP", (BPC, 128, C, KC, TY), bf16, kind="ExternalInput"
    ).ap()
    out_ap = nc.dram_tensor("out", (1, BPC * C), f32, kind="ExternalOutput").ap()

    with tile.TileContext(nc) as tc:
        with (
            tc.tile_pool(name="xt", bufs=1) as xpool,
            tc.tile_pool(name="yt", bufs=8) as ypool,
            tc.tile_pool(name="mt", bufs=1) as mpool,
            tc.tile_pool(name="ones", bufs=1) as opool,
            tc.tile_pool(name="osb", bufs=1) as obpool,
            tc.tile_pool(name="ps", bufs=7, space="PSUM") as pspool,
            tc.tile_pool(name="pso", bufs=1, space="PSUM") as psopool,
        ):
            # All of xsP for this core: (dk, b, k, t) — 3 KB/partition
            xt = xpool.tile([128, BPC, KC, TX], bf16)
            nc.sync.dma_start(xt[:], xs_ap)

            ones = opool.tile([128, 1], f32)
            nc.any.memset(ones[:], 1.0)

            # max_s scores for both batches: [t, (b, c)]
            m_all = mpool.tile([128, BPC, C], f32)

            for b in range(BPC):
                for q in range(NQ):
                    # slab: (dk, c_in_slab, k, s) — 12 KB contiguous/partition
                    yt = ypool.tile([128, QC, KC, TY], bf16)
                    nc.sync.dma_start(
                        yt[:], ys_ap[b, :, q * QC : (q + 1) * QC, :, :]
                    )
                    for g in range(QC // G):
                        ps = pspool.tile(
                            [128, G, TY], f32, name=f"ps_{b}_{q}_{g}", tag="ps"
                        )
                        for k in range(KC):
                            nc.tensor.matmul(
                                ps[:],
                                lhsT=xt[:, b, k, :],
                                rhs=yt[:, g * G : (g + 1) * G, k, :],
                                start=(k == 0),
                                stop=(k == KC - 1),
                            )
                        c0 = q * QC + g * G
                        nc.vector.reduce_max(
                            m_all[:, b, c0 : c0 + G],
                            ps[:],
                            axis=mybir.AxisListType.X,
                        )
            # sum over t (partition axis) via ones-vector matmul, fp32 exact
            out_ps = psopool.tile([1, BPC * C], f32, tag="out_ps")
            nc.tensor.matmul(
                out_ps[:], lhsT=ones[:], rhs=m_all[:], start=True, stop=True
            )
            osb = obpool.tile([1, BPC * C], f32, tag="osb")
            nc.vector.tensor_copy(osb[:], out_ps[:])
            nc.sync.dma_start(out_ap, osb[:])

    nc.compile()
    return nc


def _get_nc():
    if "nc" not in _CACHE:
        _CACHE["nc"] = _build()
    return _CACHE["nc"]


def _prep(xs: np.ndarray, ys: np.ndarray):
    """Host-side layout: bf16 cast + d-major blocked transpose."""
    import ml_dtypes

    bf16 = ml_dtypes.bfloat16
    xsb = np.asarray(xs, dtype=np.float32).astype(bf16)
    ysb = np.asarray(ys, dtype=np.float32).astype(bf16)
    # xsP[dk, b, k, t] = xs[b, t, 128k+dk]
    xsP = np.ascontiguousarray(
        xsb.reshape(B, TX, KC, 128).transpose(3, 0, 2, 1)
    )
    # ysP[b, dk, c, k, s] = ys[b, c, s, 128k+dk]
    ysP = np.ascontiguousarray(
        ysb.reshape(B, C, TY, KC, 128).transpose(0, 4, 1, 3, 2)
    )
    return xsP, ysP


def kernel(xs: np.ndarray, ys: np.ndarray) -> np.ndarray:
    from concourse.bass_utils import run_bass_kernel_spmd

    nc = _get_nc()
    xsP, ysP = _prep(xs, ys)
    in_maps = [
        {
            "xsP": np.ascontiguousarray(xsP[:, i * BPC : (i + 1) * BPC]),
            "ysP": ysP[i * BPC : (i + 1) * BPC],
        }
        for i in range(N_CORES)
    ]
    res = run_bass_kernel_spmd(nc, in_maps, core_ids=list(range(N_CORES)))
    _CACHE["last_result"] = res
    out = np.concatenate(
        [res.results[i]["out"].reshape(BPC, C) for i in range(N_CORES)], axis=0
    )
    return out.astype(np.float32)


# revision 3
# speedup vs baseline: 1.7203x; 1.7203x over previous
"""HardAttention kernel for Trainium2 (8 NeuronCores, Bass/Tile).

reference:
    scores = einsum("btd,bcsd->btcs", xs, ys)   # (B,Tx,C,Ty)
    out    = scores.max(-1).sum(1)              # (B,C)

Shapes: B=16, Tx=128, C=64, Ty=128, d=768.

Strategy (v3, fp8 + DoubleRow):
  - Data-parallel over B: core i handles batches [2i, 2i+2).
  - Host pre-casts both operands to fp8 e4m3 (inputs are N(0,1); max |v|
    ~5.4, far below the 240 clip; measured end-to-end max rel err 0.46%
    vs the 2% gate) and lays them out d-major, pre-paired for DoubleRow:
        d = 256*kk + 128*j + p   (kk in 0..2, j in 0..1, p in 0..127)
        xsP[p, b, j, kk, t]    = xs[b, t, d]      (128, B, 2, 3, Tx)
        ysP[b, p, j, c, kk, s] = ys[b, c, s, d]   (B, 128, 2, C, 3, Ty)
  - Per (b, quarter-of-64-candidates): one 1.57 MB HWDGE DMA (2x 6 KB
    contiguous runs per partition), then 3 (kk) x 4 (groups of 4
    candidates) DoubleRow matmuls (K=256, N=512) accumulating into 4
    PSUM banks; DVE reduce_max over Ty per candidate into an SBUF tile
    m[t, (b,c)]; finally one ones-vector fp32 matmul contracts the
    partition axis (sum over t) -> out[b, c].
"""

import numpy as np

B, TX, C, TY, D = 16, 128, 64, 128, 768
N_CORES = 8
BPC = B // N_CORES          # batches per core = 2
KK = D // 256               # DoubleRow contraction chunks = 3
QC = 16                     # candidates per DMA slab
NQ = C // QC                # slabs per batch = 4
G = 4                       # candidates per matmul (N = G*TY = 512)

_CACHE = {}


def _build():
    import concourse.bass as bass
    import concourse.mybir as mybir
    import concourse.tile as tile
    from concourse import bacc

    fp8 = mybir.dt.float8e4
    f32 = mybir.dt.float32
    DR = mybir.MatmulPerfMode.DoubleRow

    nc = bacc.Bacc(
        "TRN2",
        target_bir_lowering=False,
        debug=False,
        num_devices=N_CORES,
    )

    xs_ap = nc.dram_tensor(
        "xsP", (128, BPC, 2, KK, TX), fp8, kind="ExternalInput"
    ).ap()
    ys_ap = nc.dram_tensor(
        "ysP", (BPC, 128, 2, C, KK, TY), fp8, kind="ExternalInput"
    ).ap()
    out_ap = nc.dram_tensor("out", (1, BPC * C), f32, kind="ExternalOutput").ap()

    with tile.TileContext(nc) as tc:
        with (
            tc.tile_pool(name="xt", bufs=1) as xpool,
            tc.tile_pool(name="yt", bufs=6) as ypool,
            tc.tile_pool(name="mt", bufs=1) as mpool,
            tc.tile_pool(name="ones", bufs=1) as opool,
            tc.tile_pool(name="osb", bufs=1) as obpool,
            tc.tile_pool(name="ps", bufs=7, space="PSUM") as pspool,
            tc.tile_pool(name="pso", bufs=1, space="PSUM") as psopool,
        ):
            # All of xsP for this core: (p, b, j, kk, t) — 1.5 KB/partition
            xt = xpool.tile([128, BPC, 2, KK, TX], fp8)
            nc.sync.dma_start(xt[:], xs_ap)

            ones = opool.tile([128, 1], f32)
            nc.any.memset(ones[:], 1.0)

            # max_s scores for both batches: [t, (b, c)]
            m_all = mpool.tile([128, BPC, C], f32)

            for b in range(BPC):
                for q in range(NQ):
                    # slab: (p, j, c_in_slab, kk, s) — 12 KB/partition
                    yt = ypool.tile([128, 2, QC, KK, TY], fp8)
                    nc.sync.dma_start(
                        yt[:], ys_ap[b, :, :, q * QC : (q + 1) * QC, :, :]
                    )
                    psums = [
                        pspool.tile(
                            [128, G, TY], f32, name=f"ps_{b}_{q}_{g}", tag="ps"
                        )
                        for g in range(QC // G)
                    ]
                    for kk in range(KK):
                        for g in range(QC // G):
                            nc.tensor.matmul(
                                psums[g][:],
                                lhsT=xt[:, b, :, kk, :],
                                rhs=yt[:, :, g * G : (g + 1) * G, kk, :],
                                start=(kk == 0),
                                stop=(kk == KK - 1),
                                perf_mode=DR,
                            )
                    for g in range(QC // G):
                        c0 = q * QC + g * G
                        nc.vector.reduce_max(
                            m_all[:, b, c0 : c0 + G],
                            psums[g][:],
                            axis=mybir.AxisListType.X,
                        )
            # sum over t (partition axis) via ones-vector matmul, fp32 exact
            out_ps = psopool.tile([1, BPC * C], f32, tag="out_ps")
            nc.tensor.matmul(
                out_ps[:], lhsT=ones[:], rhs=m_all[:], start=True, stop=True
            )
            osb = obpool.tile([1, BPC * C], f32, tag="osb")
            nc.vector.tensor_copy(osb[:], out_ps[:])
            nc.sync.dma_start(out_ap, osb[:])

    nc.compile()
    return nc


def _get_nc():
    if "nc" not in _CACHE:
        _CACHE["nc"] = _build()
    return _CACHE["nc"]


def _prep(xs: np.ndarray, ys: np.ndarray):
    """Host-side layout: fp8 e4m3 cast + d-major DoubleRow-paired blocks."""
    import ml_dtypes

    fp8 = ml_dtypes.float8_e4m3
    xsb = np.asarray(xs, dtype=np.float32).astype(fp8)
    ysb = np.asarray(ys, dtype=np.float32).astype(fp8)
    # xsP[p, b, j, kk, t] = xs[b, t, 256kk+128j+p]
    xsP = np.ascontiguousarray(
        xsb.reshape(B, TX, KK, 2, 128).transpose(4, 0, 3, 2, 1)
    )
    # ysP[b, p, j, c, kk, s] = ys[b, c, s, 256kk+128j+p]
    ysP = np.ascontiguousarray(
        ysb.reshape(B, C, TY, KK, 2, 128).transpose(0, 5, 4, 1, 3, 2)
    )
    return xsP, ysP


def kernel(xs: np.ndarray, ys: np.ndarray) -> np.ndarray:
    from concourse.bass_utils import run_bass_kernel_spmd

    nc = _get_nc()
    xsP, ysP = _prep(xs, ys)
    in_maps = [
        {
            "xsP": np.ascontiguousarray(xsP[:, i * BPC : (i + 1) * BPC]),
            "ysP": ysP[i * BPC : (i + 1) * BPC],
        }
        for i in range(N_CORES)
    ]
    res = run_bass_kernel_spmd(nc, in_maps, core_ids=list(range(N_CORES)))
    _CACHE["last_result"] = res
    out = np.concatenate(
        [res.results[i]["out"].reshape(BPC, C) for i in range(N_CORES)], axis=0
    )
    return out.astype(np.float32)


# revision 4
# speedup vs baseline: 1.7553x; 1.0203x over previous
"""HardAttention kernel for Trainium2 (8 NeuronCores, Bass/Tile).

reference:
    scores = einsum("btd,bcsd->btcs", xs, ys)   # (B,Tx,C,Ty)
    out    = scores.max(-1).sum(1)              # (B,C)

Shapes: B=16, Tx=128, C=64, Ty=128, d=768.

Strategy (v4, fp8 + DoubleRow, tuned pipeline):
  - Data-parallel over B: core i handles batches [2i, 2i+2).
  - Host pre-casts both operands to fp8 e4m3 (inputs are N(0,1); max |v|
    ~5.4, far below the 240 clip; measured end-to-end max rel err 0.46%
    vs the 2% gate) and lays them out d-major, pre-paired for DoubleRow:
        d = 256*kk + 128*j + p   (kk in 0..2, j in 0..1, p in 0..127)
        xsP[p, b, j, kk, t]    = xs[b, t, d]      (128, B, 2, 3, Tx)
        ysP[b, p, j, c, kk, s] = ys[b, c, s, d]   (B, 128, 2, C, 3, Ty)
  - Slab DMAs alternate between the two HWDGE rings (sync + scalar) so
    descriptor generation pipelines.
  - ~16 throwaway warm-up matmuls right after xs lands keep the PE HAM
    at K=8/8 before the first real slab arrives.
  - Per (b, quarter-of-64-candidates): one 1.57 MB HWDGE DMA, then
    DoubleRow matmuls (K=256, N=512) accumulating into 4 PSUM banks,
    kk-outer so 4 matmuls share each LDWEIGHTS target; DVE reduce_max
    over Ty into an SBUF tile m[t, c]; per-batch ones-vector fp32 matmul
    contracts the partition axis (sum over t) -> out[b, c] so batch 0's
    output path completes mid-kernel.
"""

import numpy as np

B, TX, C, TY, D = 16, 128, 64, 128, 768
N_CORES = 8
BPC = B // N_CORES          # batches per core = 2
KK = D // 256               # DoubleRow contraction chunks = 3
QC = 16                     # candidates per DMA slab
NQ = C // QC                # slabs per batch = 4
G = 4                       # candidates per matmul (N = G*TY = 512)
N_WARM = 16                 # PE warm-up matmuls

_CACHE = {}


def _build():
    import concourse.bass as bass
    import concourse.mybir as mybir
    import concourse.tile as tile
    from concourse import bacc

    fp8 = mybir.dt.float8e4
    f32 = mybir.dt.float32
    DR = mybir.MatmulPerfMode.DoubleRow

    nc = bacc.Bacc(
        "TRN2",
        target_bir_lowering=False,
        debug=False,
        num_devices=N_CORES,
    )

    xs_ap = nc.dram_tensor(
        "xsP", (128, BPC, 2, KK, TX), fp8, kind="ExternalInput"
    ).ap()
    ys_ap = nc.dram_tensor(
        "ysP", (BPC, 128, 2, C, KK, TY), fp8, kind="ExternalInput"
    ).ap()
    out_ap = nc.dram_tensor("out", (BPC, C), f32, kind="ExternalOutput").ap()

    with tile.TileContext(nc) as tc:
        with (
            tc.tile_pool(name="xt", bufs=1) as xpool,
            tc.tile_pool(name="yt", bufs=6) as ypool,
            tc.tile_pool(name="mt", bufs=1) as mpool,
            tc.tile_pool(name="ones", bufs=1) as opool,
            tc.tile_pool(name="osb", bufs=2) as obpool,
            tc.tile_pool(name="ps", bufs=7, space="PSUM") as pspool,
            tc.tile_pool(name="pso", bufs=1, space="PSUM") as psopool,
        ):
            # All of xsP for this core: (p, b, j, kk, t) — 1.5 KB/partition
            xt = xpool.tile([128, BPC, 2, KK, TX], fp8)
            nc.sync.dma_start(xt[:], xs_ap)

            ones = opool.tile([128, 1], f32)
            nc.any.memset(ones[:], 1.0)

            # max_s scores: [t, (b, c)]
            m_all = mpool.tile([128, BPC, C], f32)

            # PE warm-up: throwaway DoubleRow matmuls on xs data so the HAM
            # clock-gate reaches K=8/8 before the first real slab lands.
            warm = psopool.tile([128, TX], f32, tag="pso", name="warm")
            for w in range(N_WARM):
                nc.tensor.matmul(
                    warm[:],
                    lhsT=xt[:, 0, :, w % KK, :],
                    rhs=xt[:, 0, :, (w + 1) % KK, :],
                    start=True,
                    stop=True,
                    perf_mode=DR,
                )

            dma_engines = [nc.scalar, nc.sync]
            for b in range(BPC):
                for q in range(NQ):
                    # slab: (p, j, c_in_slab, kk, s) — 12 KB/partition
                    yt = ypool.tile([128, 2, QC, KK, TY], fp8)
                    dma_engines[(b * NQ + q) % 2].dma_start(
                        yt[:], ys_ap[b, :, :, q * QC : (q + 1) * QC, :, :]
                    )
                    psums = [
                        pspool.tile(
                            [128, G, TY], f32, name=f"ps_{b}_{q}_{g}", tag="ps"
                        )
                        for g in range(QC // G)
                    ]
                    last = b == BPC - 1 and q == NQ - 1
                    if last:
                        # g-outer: each bank finishes early so its reduce
                        # overlaps the next bank's matmuls (short tail).
                        order = [
                            (kk, g) for g in range(QC // G) for kk in range(KK)
                        ]
                    else:
                        # kk-outer: 4 matmuls share each LDWEIGHTS target.
                        order = [
                            (kk, g) for kk in range(KK) for g in range(QC // G)
                        ]
                    for kk, g in order:
                        nc.tensor.matmul(
                            psums[g][:],
                            lhsT=xt[:, b, :, kk, :],
                            rhs=yt[:, :, g * G : (g + 1) * G, kk, :],
                            start=(kk == 0),
                            stop=(kk == KK - 1),
                            perf_mode=DR,
                        )
                    for g in range(QC // G):
                        c0 = q * QC + g * G
                        nc.vector.reduce_max(
                            m_all[:, b, c0 : c0 + G],
                            psums[g][:],
                            axis=mybir.AxisListType.X,
                        )
                # sum over t (partition axis) via ones-vector matmul, fp32.
                # Per batch so batch 0's output completes mid-kernel.
                out_ps = psopool.tile([1, C], f32, tag="pso", name=f"out_ps{b}")
                nc.tensor.matmul(
                    out_ps[:], lhsT=ones[:], rhs=m_all[:, b, :], start=True, stop=True
                )
                osb = obpool.tile([1, C], f32, tag="osb")
                nc.vector.tensor_copy(osb[:], out_ps[:])
                nc.sync.dma_start(out_ap[b : b + 1, :], osb[:])

    nc.compile()
    return nc


def _get_nc():
    if "nc" not in _CACHE:
        _CACHE["nc"] = _build()
    return _CACHE["nc"]


def _prep(xs: np.ndarray, ys: np.ndarray):
    """Host-side layout: fp8 e4m3 cast + d-major DoubleRow-paired blocks."""
    import ml_dtypes

    fp8 = ml_dtypes.float8_e4m3
    xsb = np.asarray(xs, dtype=np.float32).astype(fp8)
    ysb = np.asarray(ys, dtype=np.float32).astype(fp8)
    # xsP[p, b, j, kk, t] = xs[b, t, 256kk+128j+p]
    xsP = np.ascontiguousarray(
        xsb.reshape(B, TX, KK, 2, 128).transpose(4, 0, 3, 2, 1)
    )
    # ysP[b, p, j, c, kk, s] = ys[b, c, s, 256kk+128j+p]
    ysP = np.ascontiguousarray(
        ysb.reshape(B, C, TY, KK, 2, 128).transpose(0, 5, 4, 1, 3, 2)
    )
    return xsP, ysP


def kernel(xs: np.ndarray, ys: np.ndarray) -> np.ndarray:
    from concourse.bass_utils import run_bass_kernel_spmd

    nc = _get_nc()
    xsP, ysP = _prep(xs, ys)
    in_maps = [
        {
            "xsP": np.ascontiguousarray(xsP[:, i * BPC : (i + 1) * BPC]),
            "ysP": ysP[i * BPC : (i + 1) * BPC],
        }
        for i in range(N_CORES)
    ]
    res = run_bass_kernel_spmd(nc, in_maps, core_ids=list(range(N_CORES)))
    _CACHE["last_result"] = res
    out = np.concatenate(
        [res.results[i]["out"] for i in range(N_CORES)], axis=0
    )
    return out.astype(np.float32)


# revision 7
# speedup vs baseline: 1.7628x; 1.0043x over previous
"""HardAttention kernel for Trainium2 (8 NeuronCores, Bass/Tile).

reference:
    scores = einsum("btd,bcsd->btcs", xs, ys)   # (B,Tx,C,Ty)
    out    = scores.max(-1).sum(1)              # (B,C)

Shapes: B=16, Tx=128, C=64, Ty=128, d=768.

Strategy (v4, fp8 + DoubleRow, tuned pipeline):
  - Data-parallel over B: core i handles batches [2i, 2i+2).
  - Host pre-casts both operands to fp8 e4m3 (inputs are N(0,1); max |v|
    ~5.4, far below the 240 clip; measured end-to-end max rel err 0.46%
    vs the 2% gate) and lays them out d-major, pre-paired for DoubleRow:
        d = 256*kk + 128*j + p   (kk in 0..2, j in 0..1, p in 0..127)
        xsP[p, b, j, kk, t]    = xs[b, t, d]      (128, B, 2, 3, Tx)
        ysP[b, p, j, c, kk, s] = ys[b, c, s, d]   (B, 128, 2, C, 3, Ty)
  - Slab DMAs alternate between the two HWDGE rings (sync + scalar) so
    descriptor generation pipelines.
  - ~16 throwaway warm-up matmuls right after xs lands keep the PE HAM
    at K=8/8 before the first real slab arrives.
  - Per (b, quarter-of-64-candidates): one 1.57 MB HWDGE DMA, then
    DoubleRow matmuls (K=256, N=512) accumulating into 4 PSUM banks,
    kk-outer so 4 matmuls share each LDWEIGHTS target; DVE reduce_max
    over Ty into an SBUF tile m[t, c]; per-batch ones-vector fp32 matmul
    contracts the partition axis (sum over t) -> out[b, c] so batch 0's
    output path completes mid-kernel.
"""

import numpy as np

B, TX, C, TY, D = 16, 128, 64, 128, 768
N_CORES = 8
BPC = B // N_CORES          # batches per core = 2
KK = D // 256               # DoubleRow contraction chunks = 3
QC = 16                     # candidates per DMA slab
NQ = C // QC                # slabs per batch = 4
G = 4                       # candidates per matmul (N = G*TY = 512)
N_WARM = 40                 # PE warm-up matmuls (~5.6us busy, spans HAM window)

_CACHE = {}


def _build():
    import concourse.bass as bass
    import concourse.mybir as mybir
    import concourse.tile as tile
    from concourse import bacc

    fp8 = mybir.dt.float8e4
    f32 = mybir.dt.float32
    DR = mybir.MatmulPerfMode.DoubleRow

    nc = bacc.Bacc(
        "TRN2",
        target_bir_lowering=False,
        debug=False,
        num_devices=N_CORES,
    )

    xs_ap = nc.dram_tensor(
        "xsP", (128, BPC, 2, KK, TX), fp8, kind="ExternalInput"
    ).ap()
    ys_ap = nc.dram_tensor(
        "ysP", (BPC, 128, 2, C, KK, TY), fp8, kind="ExternalInput"
    ).ap()
    out_ap = nc.dram_tensor("out", (BPC, C), f32, kind="ExternalOutput").ap()

    with tile.TileContext(nc) as tc:
        with (
            tc.tile_pool(name="xt", bufs=1) as xpool,
            tc.tile_pool(name="yt", bufs=8) as ypool,
            tc.tile_pool(name="mt", bufs=1) as mpool,
            tc.tile_pool(name="ones", bufs=1) as opool,
            tc.tile_pool(name="osb", bufs=2) as obpool,
            tc.tile_pool(name="ps", bufs=7, space="PSUM") as pspool,
            tc.tile_pool(name="pso", bufs=1, space="PSUM") as psopool,
        ):
            # All of xsP for this core: (p, b, j, kk, t) — 1.5 KB/partition
            xt = xpool.tile([128, BPC, 2, KK, TX], fp8)
            nc.sync.dma_start(xt[:], xs_ap)

            ones = opool.tile([128, 1], f32)
            nc.any.memset(ones[:], 1.0)

            # max_s scores: [t, (b, c)]
            m_all = mpool.tile([128, BPC, C], f32)

            # PE warm-up: throwaway DoubleRow matmuls on xs data so the HAM
            # clock-gate reaches K=8/8 before the first real slab lands.
            warm = psopool.tile([128, TX], f32, tag="pso", name="warm")
            for w in range(N_WARM):
                nc.tensor.matmul(
                    warm[:],
                    lhsT=xt[:, 0, :, w % KK, :],
                    rhs=xt[:, 0, :, (w + 1) % KK, :],
                    start=True,
                    stop=True,
                    perf_mode=DR,
                )

            dma_engines = [nc.scalar, nc.sync]
            for b in range(BPC):
                for q in range(NQ):
                    # slab: (p, j, c_in_slab, kk, s) — 12 KB/partition
                    yt = ypool.tile([128, 2, QC, KK, TY], fp8)
                    dma_engines[(b * NQ + q) % 2].dma_start(
                        yt[:], ys_ap[b, :, :, q * QC : (q + 1) * QC, :, :]
                    )
                    psums = [
                        pspool.tile(
                            [128, G, TY], f32, name=f"ps_{b}_{q}_{g}", tag="ps"
                        )
                        for g in range(QC // G)
                    ]
                    # g-outer: each bank finishes early so its reduce
                    # overlaps the next bank's matmuls and frees PSUM early.
                    order = [
                        (kk, g) for g in range(QC // G) for kk in range(KK)
                    ]
                    for kk, g in order:
                        nc.tensor.matmul(
                            psums[g][:],
                            lhsT=xt[:, b, :, kk, :],
                            rhs=yt[:, :, g * G : (g + 1) * G, kk, :],
                            start=(kk == 0),
                            stop=(kk == KK - 1),
                            perf_mode=DR,
                        )
                    for g in range(QC // G):
                        c0 = q * QC + g * G
                        nc.vector.reduce_max(
                            m_all[:, b, c0 : c0 + G],
                            psums[g][:],
                            axis=mybir.AxisListType.X,
                        )
                # sum over t (partition axis) via ones-vector matmul, fp32.
                # Per batch so batch 0's output completes mid-kernel.
                out_ps = psopool.tile([1, C], f32, tag="pso", name=f"out_ps{b}")
                nc.tensor.matmul(
                    out_ps[:], lhsT=ones[:], rhs=m_all[:, b, :], start=True, stop=True
                )
                osb = obpool.tile([1, C], f32, tag="osb")
                nc.vector.tensor_copy(osb[:], out_ps[:])
                nc.sync.dma_start(out_ap[b : b + 1, :], osb[:])

    nc.compile()
    return nc


def _get_nc():
    if "nc" not in _CACHE:
        _CACHE["nc"] = _build()
    return _CACHE["nc"]


def _prep(xs: np.ndarray, ys: np.ndarray):
    """Host-side layout: fp8 e4m3 cast + d-major DoubleRow-paired blocks."""
    import ml_dtypes

    fp8 = ml_dtypes.float8_e4m3
    xsb = np.asarray(xs, dtype=np.float32).astype(fp8)
    ysb = np.asarray(ys, dtype=np.float32).astype(fp8)
    # xsP[p, b, j, kk, t] = xs[b, t, 256kk+128j+p]
    xsP = np.ascontiguousarray(
        xsb.reshape(B, TX, KK, 2, 128).transpose(4, 0, 3, 2, 1)
    )
    # ysP[b, p, j, c, kk, s] = ys[b, c, s, 256kk+128j+p]
    ysP = np.ascontiguousarray(
        ysb.reshape(B, C, TY, KK, 2, 128).transpose(0, 5, 4, 1, 3, 2)
    )
    return xsP, ysP


def kernel(xs: np.ndarray, ys: np.ndarray) -> np.ndarray:
    from concourse.bass_utils import run_bass_kernel_spmd

    nc = _get_nc()
    xsP, ysP = _prep(xs, ys)
    in_maps = [
        {
            "xsP": np.ascontiguousarray(xsP[:, i * BPC : (i + 1) * BPC]),
            "ysP": ysP[i * BPC : (i + 1) * BPC],
        }
        for i in range(N_CORES)
    ]
    res = run_bass_kernel_spmd(nc, in_maps, core_ids=list(range(N_CORES)))
    _CACHE["last_result"] = res
    out = np.concatenate(
        [res.results[i]["out"] for i in range(N_CORES)], axis=0
    )
    return out.astype(np.float32)
